# revision 31
# baseline (speedup 1.0000x reference)
"""Trainium2 Bass kernel for nn_CRF_70239895159020.

Reference (B=524288, C=70, 10 iterations):
    L = (S + S^T)/2 ; dL = diag(L) ; Q = log_softmax(logits, axis=1)
    repeat 10x:  P = 2*exp(Q)-1
                 tmp = logits + P @ L - dL*P       (L symmetric)
                 Q = log_sigmoid(2*tmp)

Reformulation (M = L with zero diagonal, c = colsum(M)):
    state := tmp2 + 2c = 2*logits + E @ M4,  M4 = 4M,  E := exp(Q)
    E_{t+1} = sigmoid(state_t - 2c),  E_0 = softmax(logits)
    output  = log_sigmoid(state_9 - 2c) = -softplus(-(state_9) + 2c)

Delta accumulation: state lives in PSUM for a chunk's entire 10-update
lifetime.  Init writes 2*logits (hi/lo fp16 identity matmuls, exact to
2^-21) + E0@M4; iterations accumulate only (E_t - E_{t-1})@M4 with
start=False/skip_group_check, one fp16 matmul pass per iteration (vs.
2-3 passes + fp32 in the old kernel).  The -2c term stays fp32 as a
per-partition ACT bias.  Telescoping keeps the E-contribution exact in
fp16: numpy emulation gives l2 ~5e-3 vs the 2e-2 gate (noise injected
at iteration t is amplified ~50x more at t=0 than t=9 — the map is a
period-2 oscillator, NOT a contraction, so all 10 iterations must run
and logits enter via the exact hi/lo pair while E0/M4/E are fp16).

Layout: state transposed [C=70, nch] in PSUM (4 banks per 2048-col
chunk, two chunks in flight lockstep = all 8 banks).  E^T is produced
directly by the ACT sigmoid from transposed PSUM, so the loop needs NO
per-iteration transposes or PSUM->SBUF copies: per iteration just
ACT sigmoid -> DVE fp16 subtract (4x mode) -> 4 matmuls.

E0^T: natural-layout softmax (exp on ACT, reduce on DVE, broadcast-mul
on the otherwise-idle Pool engine), then PE transposes staged through
the chunk's own state PSUM banks via an fp16 bitcast view, copied to
SBUF just before the init matmuls rebuild those banks.  Because the
bitcast view may be invisible to the tile dependency tracker, a tracked
f32 memset plus chain_iter_dep fences order slot-WAR -> transposes ->
copy -> init (without them the kernel intermittently NaNs on HW).

Final phase: E10 = sigmoid(tmp2_9) in fp32 (fp16 would underflow for
tmp2 < -17), then out = ln(E10 + 1e-38) = log_sigmoid directly; the
1e-38 bias bounds the underflow tail at -87.5.  Host only casts fp32.

Column convention: chunk k's column n of the transposed domain holds
batch row k*NCH + perm(n) with perm(t*128+p) = p*TPC + t (the PE
transpose order); lhi/llo are permuted to match on the host and the
output is unpermuted on the host.

ACT table sets: exp and ln share the combined natural_log_exp set (a
Bacc override narrows the picker to it); sigmoid lives in its own set.
Every ACT instruction is chained via chain_iter_dep so the tile
scheduler cannot interleave table sets: each epoch's 8 exps run
back-to-back (one load), and per pair only sigmoid<->ln switches
remain (~40 loads, 52 us).

Measured (real inputs, HW execute + cost-model timing): l2 7.21e-3
(gate 2e-2), 851 us/core vs 1382 us baseline.  ACT-bound: 702 us of
activation passes (12 per chunk: exp + 9 sigmoid + E10 sigmoid + ln)
+ 52 us table loads; PE/DVE/DMA all fit underneath.
"""

import os
import numpy as np

B = 524288
C = 70
N_CORES = 8
B_CORE = B // N_CORES
ITERS = 10

NCH = int(os.environ.get("KERNEL_NCH", "2048"))     # columns per chunk
EPOCH = int(os.environ.get("KERNEL_EPOCH", "8"))    # chunks per exp-batch

_prog_cache = {}
LAST_RESULTS = None


def build_program(b_core=B_CORE, nch=NCH, epoch=EPOCH):
    import concourse.bass as bass
    import concourse.bacc as bacc
    import concourse.tile as tile
    from concourse import mybir
    from contextlib import ExitStack

    f32 = mybir.dt.float32
    f16 = mybir.dt.float16
    AF = mybir.ActivationFunctionType
    Alu = mybir.AluOpType

    assert b_core % nch == 0
    nchunks = b_core // nch
    tpc = nch // 128
    assert nch % 512 == 0
    nbank = nch // 512                 # psum banks per chunk state
    assert nchunks % 2 == 0

    class _Bacc(bacc.Bacc):
        # Prefer the combined exp+ln ACT table set so the per-pair Ln
        # passes and the per-epoch Exp batch share one resident table.
        def insert_act_table_loads(self):
            from concourse.hw_specs import get_activation_tables
            has_act = any(isinstance(i, mybir.InstActivation)
                          for b in self.main_func.blocks
                          for i in b.instructions)
            if not has_act:
                return
            tabs = get_activation_tables(self.m.arch)
            combined = "natural_log_exp_and_others"
            AFt = mybir.ActivationFunctionType
            if combined in tabs:
                tables = [(n, (fs if n == combined else fs - {AFt.Exp, AFt.Ln}))
                          for n, fs in tabs.items()]
            else:
                tables = list(tabs.items())
            import bass_rust as _br
            _br.insert_act_table_loads(self, tables)

    nc = _Bacc("TRN2", target_bir_lowering=False)

    logits_d = nc.dram_tensor("logits", [b_core, C], f32, kind="ExternalInput")
    lhi_d = nc.dram_tensor("lhi", [b_core, C], f16, kind="ExternalInput")
    llo_d = nc.dram_tensor("llo", [b_core, C], f16, kind="ExternalInput")
    cf32_d = nc.dram_tensor("cf32", [C, 2], f32, kind="ExternalInput")
    cf16_d = nc.dram_tensor("cf16", [C, 140], f16, kind="ExternalInput")
    idh_d = nc.dram_tensor("idh", [128, 128], f16, kind="ExternalInput")
    out_d = nc.dram_tensor("out", [b_core, C], f16, kind="ExternalOutput")

    # natural-layout fat-descriptor views: row = k*nch + p*tpc + t
    lg = logits_d[:, :].rearrange("(k p t) c -> k p t c", p=128, t=tpc)

    with tile.TileContext(nc) as tc, ExitStack() as ctx:
        const = ctx.enter_context(tc.tile_pool(name="const", bufs=1))
        natp = ctx.enter_context(tc.tile_pool(name="nat", bufs=3))
        e0tp = ctx.enter_context(tc.tile_pool(name="e0t", bufs=epoch + 6))
        lgtp = ctx.enter_context(tc.tile_pool(name="lgt", bufs=4))
        ep = ctx.enter_context(tc.tile_pool(name="e", bufs=2))
        dp = ctx.enter_context(tc.tile_pool(name="d", bufs=2))
        outp = ctx.enter_context(tc.tile_pool(name="outp", bufs=3))
        smallp = ctx.enter_context(tc.tile_pool(name="small", bufs=3))
        psp = ctx.enter_context(tc.tile_pool(name="ps", bufs=2, space="PSUM"))

        cf32 = const.tile([C, 2], f32)
        nc.sync.dma_start(out=cf32, in_=cf32_d[:, :])
        cf16 = const.tile([C, 140], f16)
        nc.sync.dma_start(out=cf16, in_=cf16_d[:, :])
        m4h = cf16[:, 0:C]          # fp16(4M)
        id2h = cf16[:, C:2 * C]     # 2*I fp16
        idh = const.tile([128, 128], f16)
        nc.sync.dma_start(out=idh, in_=idh_d[:, :])
        b2n = cf32[:, 0:1]          # -2c fp32 (sigmoid bias)
        lnb = cf32[:, 1:2]          # 1e-38 fp32 (ln underflow bias)
        tc.strict_bb_all_engine_barrier()

        def act(*args, **kwargs):
            # chain every ACT instruction: forces the scheduler to keep
            # the emitted ACT order, so table-set switches stay batched
            i = nc.scalar.activation(*args, **kwargs)
            tc.chain_iter_dep("actorder", i.ins)
            return i

        def dma_a(k):
            natk = natp.tile([128, tpc, C], f32, tag="nat", bufs=11)
            nc.sync.dma_start(out=natk, in_=lg[k])
            return natk

        import itertools as _it
        _ctr = _it.count()

        def phase_a(k, natk):
            # natural softmax -> fp16 E0 -> DRAM roundtrip -> E0^T in SBUF
            act(natk, natk, AF.Exp)
            # big softmax passes on the otherwise-idle Pool engine so they
            # never block the DVE iteration stream
            s_t = smallp.tile([128, tpc], f32, tag="s")
            nc.vector.reduce_sum(out=s_t, in_=natk, axis=mybir.AxisListType.X)
            r_t = smallp.tile([128, tpc], f32, tag="r")
            nc.vector.reciprocal(out=r_t, in_=s_t)
            t1 = smallp.tile([128, tpc], f32, tag="t1")
            nc.vector.tensor_mul(out=t1, in0=s_t, in1=r_t)
            nc.vector.tensor_scalar(out=t1, in0=t1, scalar1=-1.0, scalar2=2.0,
                                    op0=Alu.mult, op1=Alu.add)
            nc.vector.tensor_mul(out=r_t, in0=r_t, in1=t1)
            r_bcast = bass.AP(
                tensor=r_t.tensor, offset=r_t.offset,
                ap=[r_t.ap[0], r_t.ap[1], [0, C]])
            e0nat = natp.tile([128, tpc, C], f16, tag="e0nat", bufs=14)
            nc.gpsimd.tensor_mul(out=e0nat, in0=natk, in1=r_bcast)
            return e0nat

        def load_lgt(k):
            rows = slice(k * nch, (k + 1) * nch)
            lhiT = lgtp.tile([C, nch], f16, tag="lhiT")
            nc.sync.dma_start(out=lhiT, in_=lhi_d[rows, :].rearrange("a b -> b a"))
            lloT = lgtp.tile([C, nch], f16, tag="lloT")
            nc.sync.dma_start(out=lloT, in_=llo_d[rows, :].rearrange("a b -> b a"))
            return lhiT, lloT

        def init_chunk(e0nat, lhiT, lloT):
            # E0^T via PE transposes staged through the (pre-init) state
            # psum banks, then state = 2*logits^T + E0@M4 per 512-col bank
            st = psp.tile([C, nch], f32, tag="state")
            stf16 = st.bitcast(f16)
            # ordering fence: the bitcast f16 view may be invisible to the
            # tile dep tracker, so (a) touch the tile through a tracked f32
            # write first (inherits the pool WAR on slot reuse), and (b)
            # chain transposes -> copy -> first init matmul explicitly.
            ck = f"stord{next(_ctr)}"
            gate = nc.vector.memset(st[:, 0:1], 0.0)
            tc.chain_iter_dep(ck, gate.ins)
            for s in range(tpc):
                mm = nc.tensor.matmul(
                    stf16[:, s * 128:(s + 1) * 128], lhsT=e0nat[:, s, :],
                    rhs=idh, is_transpose=True, start=True, stop=True)
                if s == 0 or s == tpc - 1:
                    tc.chain_iter_dep(ck, mm.ins)
            e0T = e0tp.tile([C, nch], f16, tag="e0T", bufs=4)
            cp = nc.vector.tensor_copy(out=e0T, in_=stf16[:, 0:nch])
            tc.chain_iter_dep(ck, cp.ins)
            first_init = [True]
            for j in range(nbank):
                sl = slice(j * 512, (j + 1) * 512)
                mm0 = nc.tensor.matmul(st[:, sl], lhsT=id2h, rhs=lhiT[:, sl],
                                        start=True, stop=False,
                                        skip_group_check=True)
                if first_init[0]:
                    tc.chain_iter_dep(ck, mm0.ins)
                    first_init[0] = False
                nc.tensor.matmul(st[:, sl], lhsT=id2h, rhs=lloT[:, sl],
                                 start=False, stop=False,
                                 skip_group_check=True)
                nc.tensor.matmul(st[:, sl], lhsT=m4h, rhs=e0T[:, sl],
                                 start=False, stop=False,
                                 skip_group_check=True)
            return st, e0T

        def make_echunk():
            p0 = ep.tile([C, nch], f16, tag="ep0")
            p1 = ep.tile([C, nch], f16, tag="ep1")
            return p0, p1

        def iter_round(st, it, eprev, ecur):
            # E_t = sigmoid(state - 2c); D = E_t - E_{t-1}; state += D @ M4
            last = it == ITERS - 1
            act(ecur, st, AF.Sigmoid, bias=b2n, scale=1.0)
            d_t = dp.tile([C, nch], f16, tag="d")
            nc.vector.tensor_sub(out=d_t, in0=ecur, in1=eprev)
            for j in range(nbank):
                sl = slice(j * 512, (j + 1) * 512)
                nc.tensor.matmul(st[:, sl], lhsT=m4h, rhs=d_t[:, sl],
                                 start=False, stop=last,
                                 skip_group_check=True)

        def finish_sig(st):
            # E10 = sigmoid(tmp2_9) in fp32 (fp16 would underflow for
            # tmp2 < -17 and wreck the ln)
            e10 = outp.tile([C, nch], f32, tag="e10", bufs=2)
            act(e10, st, AF.Sigmoid, bias=b2n, scale=1.0)
            return e10

        def finish_ln(e10, k):
            # out^T = ln(E10 + 1e-38) = log_sigmoid(tmp2_9); the bias
            # bounds the FTZ/underflow tail at ln(1e-38) = -87.5
            outT = outp.tile([C, nch], f16, tag="outT")
            act(outT, e10, AF.Ln, bias=lnb, scale=1.0)
            rows = slice(k * nch, (k + 1) * nch)
            nc.sync.dma_start(out=out_d[rows, :].rearrange("a b -> b a"),
                              in_=outT)

        nepoch = (nchunks + epoch - 1) // epoch
        e0nats = {}
        nats = {}
        lgts = {}

        def ensure_lgt(k):
            if 0 <= k < nchunks and k not in lgts:
                lgts[k] = load_lgt(k)

        for k in range(min(epoch, nchunks)):
            nats[k] = dma_a(k)
        for k in range(min(2, nchunks)):
            e0nats[k] = phase_a(k, nats.pop(k))
        for k in range(4):
            ensure_lgt(k)

        for e in range(nepoch):
            ch0 = e * epoch
            chn = min(epoch, nchunks - ch0)
            for pi in range(chn // 2):
                kA = ch0 + 2 * pi
                kB = kA + 1
                ensure_lgt(kA)
                ensure_lgt(kB)
                stA, e0A = init_chunk(e0nats.pop(kA), *lgts.pop(kA))
                stB, e0B = init_chunk(e0nats.pop(kB), *lgts.pop(kB))
                pA = make_echunk()
                pB = make_echunk()
                # iteration 1 subtracts E0 (the same fp16 tile the init
                # matmul consumed, so the telescoping is exact)
                for it in range(1, ITERS):
                    prevA = e0A if it == 1 else pA[it % 2]
                    prevB = e0B if it == 1 else pB[it % 2]
                    iter_round(stA, it, prevA, pA[(it - 1) % 2])
                    iter_round(stB, it, prevB, pB[(it - 1) % 2])
                    if it == 1:
                        ensure_lgt(kB + 1)
                        ensure_lgt(kB + 2)
                        if pi == 0:
                            for kk in range(ch0 + epoch,
                                            min(ch0 + 2 * epoch, nchunks)):
                                if kk not in nats:
                                    nats[kk] = dma_a(kk)
                    if it == 3 and e == 0 and pi == 0:
                        # rest of startup softmaxes (exp batch)
                        for kk in range(2, min(epoch, nchunks)):
                            e0nats[kk] = phase_a(kk, nats.pop(kk))
                    if it == 6 and pi == 0:
                        # next epoch's softmax batch (one exp table load),
                        # early enough that its exp->mul->transpose chain
                        # finishes before the epoch boundary
                        for kk in range(ch0 + epoch,
                                        min(ch0 + 2 * epoch, nchunks)):
                            e0nats[kk] = phase_a(kk, nats.pop(kk))
                e10A = finish_sig(stA)
                e10B = finish_sig(stB)
                # Ln passes grouped so the exp+ln table set loads once
                finish_ln(e10A, kA)
                finish_ln(e10B, kB)

    nc.compile()
    return nc


def _perm_rows(a, nch):
    # row' = k*nch + t*128 + p  <-  row k*nch + p*tpc + t  (transpose order)
    b, c = a.shape
    tpc = nch // 128
    v = a.reshape(b // nch, 128, tpc, c).transpose(0, 2, 1, 3)
    return np.ascontiguousarray(v.reshape(b, c))


def _unperm_rows(a, nch):
    b, c = a.shape
    tpc = nch // 128
    v = a.reshape(b // nch, tpc, 128, c).transpose(0, 2, 1, 3)
    return np.ascontiguousarray(v.reshape(b, c))


def _host_prep(logits, similarities):
    S = np.asarray(similarities, dtype=np.float32)
    L = (S + S.T) * np.float32(0.5)
    M = L.copy()
    np.fill_diagonal(M, 0.0)
    m4 = (4.0 * M).astype(np.float32)
    col = M.astype(np.float64).sum(axis=0)
    cf32 = np.zeros((C, 2), dtype=np.float32)
    cf32[:, 0] = (-2.0 * col).astype(np.float32)
    cf32[:, 1] = np.float32(1e-38)
    cf16 = np.zeros((C, 140), dtype=np.float16)
    cf16[:, 0:C] = m4.astype(np.float16)
    cf16[:, C:2 * C] = (2.0 * np.eye(C)).astype(np.float16)
    idh = np.eye(128, dtype=np.float16)
    lhi = logits.astype(np.float16)
    llo = (logits - lhi.astype(np.float32)).astype(np.float16)
    lhi = _perm_rows(lhi, NCH)
    llo = _perm_rows(llo, NCH)
    return cf32, cf16, lhi, llo, idh


def kernel(logits, similarities):
    global LAST_RESULTS
    from concourse.bass_utils import run_bass_kernel_spmd

    logits = np.ascontiguousarray(np.asarray(logits), dtype=np.float32)
    cf32, cf16, lhi, llo, idh = _host_prep(logits, similarities)

    key = (B_CORE, NCH, EPOCH)
    if key not in _prog_cache:
        _prog_cache[key] = build_program()
    nc = _prog_cache[key]

    shards = logits.reshape(N_CORES, B_CORE, C)
    lhi_s = lhi.reshape(N_CORES, B_CORE, C)
    llo_s = llo.reshape(N_CORES, B_CORE, C)
    in_maps = []
    for i in range(N_CORES):
        in_maps.append({
            "logits": shards[i],
            "lhi": np.ascontiguousarray(lhi_s[i]),
            "llo": np.ascontiguousarray(llo_s[i]),
            "cf32": cf32, "cf16": cf16, "idh": idh,
        })
    trace = os.environ.get("KERNEL_TRACE", "0") == "1"
    res = run_bass_kernel_spmd(nc, in_maps, core_ids=list(range(N_CORES)),
                               trace=trace)
    LAST_RESULTS = res
    out = np.concatenate(
        [_unperm_rows(np.asarray(r["out"]), NCH) for r in res.results], axis=0)
    return np.ascontiguousarray(out.astype(np.float32))
